# revision 1
# baseline (speedup 1.0000x reference)
import os
import numpy as np

# Shapes are fixed by the problem spec.
BS, N, NE, D, DS, NT, NH, NL, NF = 1, 16384, 262144, 128, 32, 64, 4, 2, 2

_JIT_CACHE = {}


def _build_csr(idx, n):
    """Padded CSR: for each node, the list of edge positions with that node,
    padded with sentinel NE. Returns (csr [n, K] int32, mask [n, K] float32)."""
    ne = idx.shape[0]
    order = np.argsort(idx, kind='stable')
    sidx = idx[order]
    counts = np.bincount(idx, minlength=n)
    K = int(counts.max())
    rowptr = np.zeros(n + 1, dtype=np.int64)
    np.cumsum(counts, out=rowptr[1:])
    csr = np.full((n, K), ne, dtype=np.int32)
    ar = np.arange(ne)
    # position of each sorted edge within its node's run
    col = ar - rowptr[sidx]
    csr[sidx, col] = order.astype(np.int32)
    mask = (csr != ne).astype(np.float32)
    return csr, mask


def _model(jnp, jax):
    """Builds the jitted model function (imports jax lazily)."""

    def silu(x):
        return x * jax.nn.sigmoid(x)

    def lin(x, p):
        return x @ p['w'].T + p['b']

    def layernorm(x, p):
        m = x.mean(-1, keepdims=True)
        v = ((x - m) ** 2).mean(-1, keepdims=True)
        return (x - m) / jnp.sqrt(v + 1e-5) * p['g'] + p['b']

    def rmsnorm(x):
        return x / jnp.sqrt(jnp.mean(x * x, axis=-1, keepdims=True) + 1e-8)

    def mlp(x, p):
        y = lin(silu(lin(x, p['l1'])), p['l2'])
        return layernorm(y, p['ln']) if 'ln' in p else y

    def attn_head(x, p):
        return lin(silu(lin(x, p['l1'])), p['l2'])[..., 0]

    def mha(q, k, v, p, nh):
        b, lq, d = q.shape
        hd = d // nh
        Q = lin(q, p['q']).reshape(b, lq, nh, hd)
        K = lin(k, p['k']).reshape(b, k.shape[1], nh, hd)
        V = lin(v, p['v']).reshape(b, v.shape[1], nh, hd)
        att = jax.nn.softmax(
            jnp.einsum('bqhd,bkhd->bhqk', Q, K) / np.sqrt(hd).astype(np.float32),
            axis=-1)
        o = jnp.einsum('bhqk,bkhd->bqhd', att, V).reshape(b, lq, d)
        return lin(o, p['o'])

    def block_attn_res(blocks, partial, w):
        V = jnp.stack(list(blocks) + [partial], axis=0)
        logits = jnp.clip(jnp.einsum('d,sbnd->sbn', w, rmsnorm(V)), -30, 30)
        alpha = jax.nn.softmax(logits, axis=0)
        return jnp.einsum('sbn,sbnd->bnd', alpha, V)

    def csr_aggregate(logit, msg, csr, mask):
        """agg[n] = sum_e exp(l_e - m_n) msg_e / (s_n + 1e-16) over edges of node n."""
        l_ext = jnp.concatenate([logit, jnp.full((1,), -1e30, jnp.float32)])
        lg = l_ext[csr]                              # [N, K]
        m = lg.max(axis=1)                           # [N]
        e = jnp.exp(lg - m[:, None]) * mask          # [N, K]
        s = e.sum(axis=1)                            # [N]
        msg_ext = jnp.concatenate([msg, jnp.zeros((1, msg.shape[1]), jnp.float32)])
        mg = msg_ext[csr]                            # [N, K, 64]
        agg = jnp.einsum('nk,nkd->nd', e, mg)
        return agg / (s[:, None] + 1e-16)

    def gnn_apply(V_in, Ef, si, ri, csr_s, mask_s, csr_r, mask_r, p):
        v = V_in[0]                                  # [N, ns]
        senders = v[si]                              # [NE, ns]
        receivers = v[ri]
        x = jnp.concatenate([senders, receivers, Ef[0]], axis=-1)
        ee = mlp(x, p['f_edge'])                     # [NE, D]
        ms = mlp(ee, p['f_msg_s'])                   # [NE, D//2]
        mr = mlp(ee, p['f_msg_r'])
        ls = jnp.clip(attn_head(ee, p['f_attn_s']), -30, 30)   # [NE]
        lr = jnp.clip(attn_head(ee, p['f_attn_r']), -30, 30)
        agg0 = csr_aggregate(ls, ms, csr_s, mask_s)  # [N, D//2]
        agg1 = csr_aggregate(lr, mr, csr_r, mask_r)
        node = mlp(jnp.concatenate([v, agg0, agg1], axis=-1), p['f_node'])
        return node[None], ee[None]

    def atten_apply(W0, p):
        q = jnp.broadcast_to(p['Q'][None], (W0.shape[0],) + p['Q'].shape) + \
            lin(silu(lin(W0.mean(axis=1, keepdims=True), p['qo1'])), p['qo2'])
        W = mha(q, W0, W0, p['a1'], NH)
        for lp_ in p['a2']:
            W = mha(W, W, W, lp_, NH)
        return mha(W0, W, W, p['a3'], NH)

    def cross_apply(Vs, Vo, p):
        Q = jnp.broadcast_to(p['Q'][None], (Vs.shape[0],) + p['Q'].shape)
        other = layernorm(Vo, p['ln_o'])
        sn = layernorm(Vs, p['ln_s'])
        W = mha(Q, other, other, p['a1'], NH)
        W = mha(W, W, W, p['a2'], NH)
        return mha(sn, W, W, p['a3'], NH)

    def run(V0, V1, E0, E1, si, ri, csr_s, mask_s, csr_r, mask_r,
            s_enc, B00, B01, B10, B11, params):
        V_list = [V0, V1]
        E_list = [E0, E1]
        blocks = [[B00, B01], [B10, B11]]
        w = params['attn_res_w']
        V_out, E_out = [], []
        for i in range(NF):
            h = block_attn_res(blocks[i], V_list[i], w[3 * i + 0])
            V_in = jnp.concatenate([h, s_enc], axis=-1)
            v, e = gnn_apply(V_in, E_list[i], si, ri, csr_s, mask_s,
                             csr_r, mask_r, params['gnn'][i])
            V_out.append(v)
            E_out.append(E_list[i] + e)
        cross = [cross_apply(V_out[i], V_out[1 - i], params['cross'][i])
                 for i in range(NF)]
        outs = []
        for i in range(NF):
            partial = V_out[i] + cross[i]
            h = block_attn_res(blocks[i], partial, w[3 * i + 1])
            partial = partial + atten_apply(layernorm(h, params['ln1'][i]),
                                            params['mha'][i])
            h = block_attn_res(blocks[i], partial, w[3 * i + 2])
            hn = layernorm(h, params['ln2'][i])
            y = lin(silu(lin(hn, params['ffn'][i]['l1'])),
                    params['ffn'][i]['l2'])
            outs.append(partial + y)
        return (outs[0], outs[1], E_out[0], E_out[1])

    return run


def kernel(V0, V1, E0, E1, edges, s_enc, B00, B01, B10, B11, params):
    import jax
    import jax.numpy as jnp

    edges_np = np.asarray(edges)
    si = edges_np[0, :, 0].astype(np.int32)
    ri = edges_np[0, :, 1].astype(np.int32)
    csr_s, mask_s = _build_csr(si, N)
    csr_r, mask_r = _build_csr(ri, N)

    key = ('run', csr_s.shape[1], csr_r.shape[1])
    if key not in _JIT_CACHE:
        _JIT_CACHE[key] = jax.jit(_model(jnp, jax))
    run = _JIT_CACHE[key]

    out = run(jnp.asarray(V0), jnp.asarray(V1), jnp.asarray(E0),
              jnp.asarray(E1), jnp.asarray(si), jnp.asarray(ri),
              jnp.asarray(csr_s), jnp.asarray(mask_s), jnp.asarray(csr_r),
              jnp.asarray(mask_r), jnp.asarray(s_enc), jnp.asarray(B00),
              jnp.asarray(B01), jnp.asarray(B10), jnp.asarray(B11),
              jax.tree.map(jnp.asarray, params))
    return tuple(np.asarray(o) for o in out)
